# revision 20
# baseline (speedup 1.0000x reference)
"""BatchAllTripletLoss TRN2 kernel — v3.

Per core: tiles of 128 pairs x 512 negatives.  PE builds full d^2 (+BIG
mask) in PSUM via bf16 matmuls (Gram + one-hot mask with device-written
||e_n||^2 / ||e_a||^2 contraction rows).  ACT sqrts two tiles per pass
and accumulates half the loss (Relu) and half the counts (Sign, squared
domain, straight from PSUM); DVE accumulates the rest
(scalar_tensor_tensor min / is_lt cache-reduce).  Host does label-index
prep, one-hot mask operands, num_valid, and the final scalar division.
"""

import math

import numpy as np
import ml_dtypes

import concourse.bass as bass
import concourse.tile as tile
from concourse import bacc, mybir
from concourse.bass_utils import run_bass_kernel_spmd

B = 512
D = 128
NCORES = 8
MARGIN = 0.2
BIG = float(2 ** 100)

F32 = mybir.dt.float32
BF16 = mybir.dt.bfloat16
AF = mybir.ActivationFunctionType
OP = mybir.AluOpType
BF = ml_dtypes.bfloat16

TRACE = False
LAST_RESULT = None
_PROGRAM_CACHE = {}
NAROW = 64     # alohx device row: ||e_a||^2   (paired with lohx ones row)
NNROW = 96     # lohx device row: ||e_n||^2    (paired with alohx ones row)


def _build_program(n_tiles: int):
    npc = n_tiles * 128
    nc = bacc.Bacc("TRN2", target_bir_lowering=False, debug=False)
    fe = B + 32
    embT_d = nc.dram_tensor("embT", [128, fe], BF16, kind="ExternalInput")
    am2T_d = nc.dram_tensor("am2T", [128, npc], BF16, kind="ExternalInput")
    ap_d = nc.dram_tensor("ap_emb", [128, 2 * npc], BF16,
                          kind="ExternalInput")
    mask_d = nc.dram_tensor("mask", [96, npc + B], BF16, kind="ExternalInput")
    out_d = nc.dram_tensor("out", [128, 2 * n_tiles], F32,
                           kind="ExternalOutput")

    with tile.TileContext(nc) as tc:
        from contextlib import ExitStack

        with ExitStack() as ctx:
            _body(ctx, tc, n_tiles, embT_d, am2T_d, ap_d, mask_d, out_d)
    nc.compile()
    return nc


def _body(ctx, tc, n_tiles, embT_d, am2T_d, ap_d, mask_d, out_d):
    nc = tc.nc
    npc = n_tiles * 128
    fe = B + 32
    ngrp = (n_tiles + 1) // 2

    const = ctx.enter_context(tc.tile_pool(name="const", bufs=1))
    work = ctx.enter_context(tc.tile_pool(name="work", bufs=3))
    small = ctx.enter_context(tc.tile_pool(name="small", bufs=4))
    psq = ctx.enter_context(tc.tile_pool(name="psq", bufs=2, space="PSUM"))
    psum1 = ctx.enter_context(tc.tile_pool(name="psum1", bufs=1, space="PSUM"))

    # force both ACT table loads (sqrt set) at kernel start, overlapped
    # with the DMA wait; Copy/Relu/Sign all ride in the same set
    dumm = small.tile([128, 2], F32, tag="dumm")
    nc.vector.memset(dumm, 1.0)
    dummo = small.tile([128, 2], F32, tag="dummo")
    nc.scalar.activation(dummo, dumm, AF.Sqrt)

    # DMAs: each dma_start costs ~2us fixed completion latency; the
    # three big tensors share the sync HWDGE ring (issue-ordered by
    # need), embT rides scalar so its ring stays short
    embTt = const.tile([128, fe], BF16)
    nc.scalar.dma_start(out=embTt, in_=embT_d.ap())
    am2T = const.tile([128, npc], BF16)
    nc.sync.dma_start(out=am2T, in_=am2T_d.ap())
    maskt = const.tile([97, npc + B], BF16)
    nc.sync.dma_start(out=maskt[0:96, :], in_=mask_d.ap())
    ap_emb = const.tile([128, 2 * npc], BF16)
    nc.gpsimd.dma_start(out=ap_emb, in_=ap_d.ap())

    embT = embTt[:, 0:B]
    lhs_nn = embTt[:, B:B + 1]        # ones
    lhs_na = embTt[:, B + 3:B + 4]    # 0.25s
    a_emb = ap_emb[:, 0:npc]
    p_emb = ap_emb[:, npc:2 * npc]
    alohx = maskt[:, 0:npc]
    lohx = maskt[:, npc:npc + B]

    # alohx ones row (96) is memset on device; rows 66-95 ship as zeros
    nc.vector.memset(alohx[NNROW:NNROW + 1, :], 1.0)
    zeros_b = const.tile([128, B], BF16)
    nc.vector.memset(zeros_b, 0.0)

    # PE clock warm-up during the DMA wait: the HAM gate needs ~2.5us of
    # activity to reach full rate; single-partition operands keep the
    # SBUF ports free for the streaming input DMAs
    wz = small.tile([1, B], BF16, tag="wz")
    nc.vector.memset(wz, 0.0)
    pwu = psum1.tile([1, B], F32, tag="pwu")
    for _ in range(5):
        nc.tensor.matmul(pwu, lhsT=wz[:, 0:1], rhs=wz, start=True,
                         stop=True)

    # ---- norms: ||e_n||^2 -> lohx row 96, ||e_a||^2 -> alohx row 64
    embT_sq = work.tile([128, B], BF16, tag="embT_sq")
    nc.vector.tensor_mul(embT_sq, embT, embT)
    am2T_sq = work.tile([128, npc], BF16, tag="am2T_sq")
    nc.vector.tensor_mul(am2T_sq, am2T, am2T)
    psum_nn = psum1.tile([1, B], F32, tag="pnn")
    nc.tensor.matmul(psum_nn, lhsT=lhs_nn, rhs=embT_sq, start=True,
                     stop=True)
    psum_na = psum1.tile([1, B], F32, tag="pna")
    nc.tensor.matmul(psum_na, lhsT=lhs_na, rhs=am2T_sq, start=True,
                     stop=True)
    nc.scalar.copy(lohx[NNROW:NNROW + 1, :], psum_nn)
    nc.vector.tensor_copy(alohx[NAROW:NAROW + 1, :], psum_na)

    # ---- X phase: xp = ||e_a - e_p|| + margin, xp2 = xp^2
    diff = work.tile([128, npc], BF16, tag="diff")
    nc.vector.tensor_sub(diff, a_emb, p_emb)
    dsq = work.tile([128, npc], BF16, tag="dsq")
    nc.vector.tensor_mul(dsq, diff, diff)
    xsq_cols = const.tile([128, n_tiles], F32)
    nc.vector.tensor_reduce(xsq_cols, dsq.rearrange("p (t d) -> p t d", d=D),
                            axis=mybir.AxisListType.X, op=OP.add)
    x0 = small.tile([128, n_tiles], F32, tag="x0")
    nc.scalar.activation(x0, xsq_cols, AF.Sqrt)
    xp = const.tile([128, n_tiles], F32)
    nc.vector.tensor_scalar_add(xp, x0, MARGIN)
    xp2 = const.tile([128, n_tiles], F32)
    nc.vector.tensor_mul(xp2, xp, xp)

    # ---- q matmuls, two tiles per 2-bank PSUM group
    qg = []
    for g in range(ngrp):
        gt = psq.tile([128, 2 * B], F32, tag="qg")
        qg.append(gt)
        for j in range(min(2, n_tiles - 2 * g)):
            t = 2 * g + j
            nc.tensor.matmul(gt[:, j * B:(j + 1) * B],
                             lhsT=am2T[:, bass.ts(t, 128)], rhs=embT,
                             start=True, stop=False)
    for g in range(ngrp):
        for j in range(min(2, n_tiles - 2 * g)):
            t = 2 * g + j
            nc.tensor.matmul(qg[g][:, j * B:(j + 1) * B],
                             lhsT=alohx[:, bass.ts(t, 128)], rhs=lohx,
                             start=False, stop=True)

    out_sb = small.tile([128, 2 * n_tiles], F32, tag="out_sb")

    for g in range(ngrp):
        gsz = min(2, n_tiles - 2 * g)
        Dg = work.tile([128, 2 * B], BF16, tag="Dg")
        nc.scalar.activation(Dg[:, 0:gsz * B], qg[g][:, 0:gsz * B], AF.Sqrt)
        for j in range(gsz):
            t = 2 * g + j
            Dt = Dg[:, j * B:(j + 1) * B]
            qt = qg[g][:, j * B:(j + 1) * B]
            if t < 2:
                # count on ACT: sum sign(xp^2 - q) = 2*count - 512
                scr_c = work.tile([128, B], F32, tag="scr_c")
                nc.scalar.activation(
                    scr_c, qt, AF.Sign, bias=xp2[:, t:t + 1], scale=-1.0,
                    accum_out=out_sb[:, n_tiles + t:n_tiles + t + 1])
                # loss on DVE: sum min(D - xp, 0) = -sum relu(xp - D)
                scr_l = work.tile([128, B], BF16, tag="scr_l")
                nc.vector.scalar_tensor_tensor(
                    out=scr_l, in0=Dt, scalar=xp[:, t:t + 1], in1=zeros_b,
                    op0=OP.subtract, op1=OP.min,
                    accum_out=out_sb[:, t:t + 1])
            else:
                # loss on ACT: sum relu(xp - D)
                scr_l = work.tile([128, B], BF16, tag="scr_l")
                nc.scalar.activation(
                    scr_l, Dt, AF.Relu, bias=xp[:, t:t + 1], scale=-1.0,
                    accum_out=out_sb[:, t:t + 1])
                # count on DVE: sum (D < xp)
                scr_c = work.tile([128, B], BF16, tag="scr_c")
                nc.vector.tensor_scalar(
                    scr_c, Dt, xp[:, t:t + 1], None, op0=OP.is_lt,
                    op1=OP.add,
                    accum_out=out_sb[:, n_tiles + t:n_tiles + t + 1])

    nc.sync.dma_start(out=out_d.ap(), in_=out_sb)


def _host_prepare(labels: np.ndarray, emb: np.ndarray):
    labels = np.asarray(labels).astype(np.int64)
    emb = np.ascontiguousarray(np.asarray(emb, dtype=np.float32))
    b = labels.shape[0]
    ncls = int(labels.max()) + 1
    assert ncls <= 64

    pairs_a, pairs_p = [], []
    by_class = {}
    for i, lab in enumerate(labels.tolist()):
        by_class.setdefault(lab, []).append(i)
    for idxs in by_class.values():
        for a in idxs:
            for p in idxs:
                if a != p:
                    pairs_a.append(a)
                    pairs_p.append(p)
    np_total = len(pairs_a)
    per_core = max(1, math.ceil(np_total / NCORES))
    n_tiles = max(1, math.ceil(per_core / 128))
    npc = n_tiles * 128

    m = np.bincount(labels, minlength=ncls).astype(np.int64)
    num_valid = int((m * (m - 1) * (b - m)).sum())

    sq = (emb * emb).sum(1)
    d2 = sq[:, None] + sq[None, :] - 2.0 * (emb @ emb.T)
    neq = labels[:, None] != labels[None, :]
    assert not neq.any() or d2[neq].min() > 16.0

    embT = emb.T
    onehot = (labels[None, :] ==
              np.arange(ncls)[:, None]).astype(np.float32)
    lohx = np.zeros((96, b), np.float32)
    lohx[0:ncls, :] = onehot
    lohx[NAROW, :] = 1.0          # pairs with device ||e_a||^2 row

    in_maps = []
    for k in range(NCORES):
        a_idx = pairs_a[k * per_core:(k + 1) * per_core]
        p_idx = pairs_p[k * per_core:(k + 1) * per_core]
        nreal = len(a_idx)

        am2T = np.zeros((D, npc), np.float32)
        a_emb = np.zeros((npc, D), np.float32)
        p_emb = np.zeros((npc, D), np.float32)
        alohx = np.zeros((96, npc), np.float32)
        alohx[0:ncls, :] = BIG        # pads: BIG in every class row
        if nreal:
            ga = emb[a_idx]
            am2T[:, :nreal] = (-2.0 * ga).T
            a_emb[:nreal] = ga
            p_emb[:nreal] = emb[p_idx]
            alohx[0:ncls, :nreal] = BIG * onehot[:, a_idx]

        a_emb2 = np.ascontiguousarray(
            a_emb.reshape(n_tiles, 128, D).transpose(1, 0, 2)).reshape(128, -1)
        p_emb2 = np.ascontiguousarray(
            p_emb.reshape(n_tiles, 128, D).transpose(1, 0, 2)).reshape(128, -1)
        epad = np.zeros((128, 32), np.float32)
        epad[:, 0] = 1.0              # lhs_nn ones
        epad[:, 3] = 0.25             # lhs_na quarters
        embT_t = np.concatenate([embT, epad], axis=1)
        maskc = np.concatenate([alohx, lohx], axis=1)
        ap_c = np.concatenate([a_emb2, p_emb2], axis=1)
        in_maps.append({
            "embT": np.ascontiguousarray(embT_t).astype(BF),
            "am2T": np.ascontiguousarray(am2T).astype(BF),
            "ap_emb": np.ascontiguousarray(ap_c).astype(BF),
            "mask": np.ascontiguousarray(maskc).astype(BF),
        })
    return in_maps, n_tiles, num_valid


def kernel(labels: np.ndarray, embeddings: np.ndarray):
    global LAST_RESULT
    in_maps, n_tiles, num_valid = _host_prepare(labels, embeddings)

    if n_tiles not in _PROGRAM_CACHE:
        _PROGRAM_CACHE[n_tiles] = _build_program(n_tiles)
    nc = _PROGRAM_CACHE[n_tiles]

    res = run_bass_kernel_spmd(nc, in_maps, list(range(NCORES)), trace=TRACE)
    LAST_RESULT = res

    outs = np.stack([np.asarray(r["out"], np.float64) for r in res.results])
    nact = min(2, n_tiles)
    # loss: tiles 0..1 accumulate -sum relu (DVE), 2.. +sum relu (ACT);
    # counts: tiles 0..1 sign-coded (2c-512), the rest direct
    s_sum = (-outs[:, :, 0:nact].sum()) + outs[:, :, nact:n_tiles].sum()
    csign = outs[:, :, n_tiles:n_tiles + nact]
    c_sum = ((csign + 512.0) / 2.0).sum() + \
        outs[:, :, n_tiles + nact:2 * n_tiles].sum()
    loss = np.float32(s_sum / (c_sum + 1e-16))
    frac = np.float32(c_sum / (num_valid + 1e-16))
    return (np.asarray(loss, np.float32), np.asarray(frac, np.float32))


# revision 21
# speedup vs baseline: 1.0058x; 1.0058x over previous
"""BatchAllTripletLoss TRN2 kernel — v3.

Per core: tiles of 128 pairs x 512 negatives.  PE builds full d^2 (+BIG
mask) in PSUM via bf16 matmuls (Gram + one-hot mask with device-written
||e_n||^2 / ||e_a||^2 contraction rows).  ACT sqrts two tiles per pass
and accumulates half the loss (Relu) and half the counts (Sign, squared
domain, straight from PSUM); DVE accumulates the rest
(scalar_tensor_tensor min / is_lt cache-reduce).  Host does label-index
prep, one-hot mask operands, num_valid, and the final scalar division.
"""

import math

import numpy as np
import ml_dtypes

import concourse.bass as bass
import concourse.tile as tile
from concourse import bacc, mybir
from concourse.bass_utils import run_bass_kernel_spmd

B = 512
D = 128
NCORES = 8
MARGIN = 0.2
BIG = float(2 ** 100)

F32 = mybir.dt.float32
BF16 = mybir.dt.bfloat16
AF = mybir.ActivationFunctionType
OP = mybir.AluOpType
BF = ml_dtypes.bfloat16

TRACE = False
LAST_RESULT = None
_PROGRAM_CACHE = {}
NAROW = 64     # alohx device row: ||e_a||^2   (paired with lohx ones row)
NNROW = 96     # lohx device row: ||e_n||^2    (paired with alohx ones row)


def _build_program(n_tiles: int):
    npc = n_tiles * 128
    nc = bacc.Bacc("TRN2", target_bir_lowering=False, debug=False)
    fe = B + 32
    embT_d = nc.dram_tensor("embT", [128, fe], BF16, kind="ExternalInput")
    am2T_d = nc.dram_tensor("am2T", [128, npc], BF16, kind="ExternalInput")
    ap_d = nc.dram_tensor("ap_emb", [128, 2 * npc], BF16,
                          kind="ExternalInput")
    mask_d = nc.dram_tensor("mask", [96, npc + B], BF16, kind="ExternalInput")
    out_d = nc.dram_tensor("out", [128, 2 * n_tiles], F32,
                           kind="ExternalOutput")

    with tile.TileContext(nc) as tc:
        from contextlib import ExitStack

        with ExitStack() as ctx:
            _body(ctx, tc, n_tiles, embT_d, am2T_d, ap_d, mask_d, out_d)
    nc.compile()
    return nc


def _body(ctx, tc, n_tiles, embT_d, am2T_d, ap_d, mask_d, out_d):
    nc = tc.nc
    npc = n_tiles * 128
    fe = B + 32
    ngrp = (n_tiles + 1) // 2

    const = ctx.enter_context(tc.tile_pool(name="const", bufs=1))
    work = ctx.enter_context(tc.tile_pool(name="work", bufs=3))
    small = ctx.enter_context(tc.tile_pool(name="small", bufs=4))
    psq = ctx.enter_context(tc.tile_pool(name="psq", bufs=2, space="PSUM"))
    psum1 = ctx.enter_context(tc.tile_pool(name="psum1", bufs=1, space="PSUM"))

    # force both ACT table loads (sqrt set) at kernel start, overlapped
    # with the DMA wait; Copy/Relu/Sign all ride in the same set
    dumm = small.tile([128, 2], F32, tag="dumm")
    nc.vector.memset(dumm, 1.0)
    dummo = small.tile([128, 2], F32, tag="dummo")
    nc.scalar.activation(dummo, dumm, AF.Sqrt)

    # DMAs: each dma_start costs ~2us fixed completion latency; the
    # three big tensors share the sync HWDGE ring (issue-ordered by
    # need), embT rides scalar so its ring stays short
    embTt = const.tile([128, fe], BF16)
    nc.scalar.dma_start(out=embTt, in_=embT_d.ap())
    am2T = const.tile([128, npc], BF16)
    nc.sync.dma_start(out=am2T, in_=am2T_d.ap())
    maskt = const.tile([97, npc + B], BF16)
    nc.sync.dma_start(out=maskt[0:96, :], in_=mask_d.ap())
    ap_emb = const.tile([128, 2 * npc], BF16)
    nc.gpsimd.dma_start(out=ap_emb, in_=ap_d.ap())

    embT = embTt[:, 0:B]
    lhs_nn = embTt[:, B:B + 1]        # ones
    lhs_na = embTt[:, B + 3:B + 4]    # 0.25s
    a_emb = ap_emb[:, 0:npc]
    p_emb = ap_emb[:, npc:2 * npc]
    alohx = maskt[:, 0:npc]
    lohx = maskt[:, npc:npc + B]

    # alohx ones row (96) is memset on device; rows 66-95 ship as zeros
    nc.vector.memset(alohx[NNROW:NNROW + 1, :], 1.0)
    zeros_b = const.tile([128, B], BF16)
    nc.vector.memset(zeros_b, 0.0)

    # ---- norms: ||e_n||^2 -> lohx row 96, ||e_a||^2 -> alohx row 64
    embT_sq = work.tile([128, B], BF16, tag="embT_sq")
    nc.vector.tensor_mul(embT_sq, embT, embT)
    am2T_sq = work.tile([128, npc], BF16, tag="am2T_sq")
    nc.vector.tensor_mul(am2T_sq, am2T, am2T)
    psum_nn = psum1.tile([1, B], F32, tag="pnn")
    nc.tensor.matmul(psum_nn, lhsT=lhs_nn, rhs=embT_sq, start=True,
                     stop=True)
    psum_na = psum1.tile([1, B], F32, tag="pna")
    nc.tensor.matmul(psum_na, lhsT=lhs_na, rhs=am2T_sq, start=True,
                     stop=True)
    nc.scalar.copy(lohx[NNROW:NNROW + 1, :], psum_nn)
    nc.scalar.copy(alohx[NAROW:NAROW + 1, :], psum_na)

    # ---- X phase: xp = ||e_a - e_p|| + margin, xp2 = xp^2
    diff = work.tile([128, npc], BF16, tag="diff")
    nc.vector.tensor_sub(diff, a_emb, p_emb)
    dsq = work.tile([128, npc], BF16, tag="dsq")
    nc.vector.tensor_mul(dsq, diff, diff)
    xsq_cols = const.tile([128, n_tiles], F32)
    nc.vector.tensor_reduce(xsq_cols, dsq.rearrange("p (t d) -> p t d", d=D),
                            axis=mybir.AxisListType.X, op=OP.add)
    x0 = small.tile([128, n_tiles], F32, tag="x0")
    nc.scalar.activation(x0, xsq_cols, AF.Sqrt)
    xp = const.tile([128, n_tiles], F32)
    nc.vector.tensor_scalar_add(xp, x0, MARGIN)
    xp2 = const.tile([128, n_tiles], F32)
    nc.vector.tensor_mul(xp2, xp, xp)

    # ---- q matmuls, two tiles per 2-bank PSUM group
    qg = []
    for g in range(ngrp):
        gt = psq.tile([128, 2 * B], F32, tag="qg")
        qg.append(gt)
        for j in range(min(2, n_tiles - 2 * g)):
            t = 2 * g + j
            nc.tensor.matmul(gt[:, j * B:(j + 1) * B],
                             lhsT=am2T[:, bass.ts(t, 128)], rhs=embT,
                             start=True, stop=False)
    for g in range(ngrp):
        for j in range(min(2, n_tiles - 2 * g)):
            t = 2 * g + j
            nc.tensor.matmul(qg[g][:, j * B:(j + 1) * B],
                             lhsT=alohx[:, bass.ts(t, 128)], rhs=lohx,
                             start=False, stop=True)

    out_sb = small.tile([128, 2 * n_tiles], F32, tag="out_sb")

    for g in range(ngrp):
        gsz = min(2, n_tiles - 2 * g)
        Dg = work.tile([128, 2 * B], BF16, tag="Dg")
        nc.scalar.activation(Dg[:, 0:gsz * B], qg[g][:, 0:gsz * B], AF.Sqrt)
        for j in range(gsz):
            t = 2 * g + j
            Dt = Dg[:, j * B:(j + 1) * B]
            qt = qg[g][:, j * B:(j + 1) * B]
            if t < 2:
                # count on ACT: sum sign(xp^2 - q) = 2*count - 512
                scr_c = work.tile([128, B], F32, tag="scr_c")
                nc.scalar.activation(
                    scr_c, qt, AF.Sign, bias=xp2[:, t:t + 1], scale=-1.0,
                    accum_out=out_sb[:, n_tiles + t:n_tiles + t + 1])
                # loss on DVE: sum min(D - xp, 0) = -sum relu(xp - D)
                scr_l = work.tile([128, B], BF16, tag="scr_l")
                nc.vector.scalar_tensor_tensor(
                    out=scr_l, in0=Dt, scalar=xp[:, t:t + 1], in1=zeros_b,
                    op0=OP.subtract, op1=OP.min,
                    accum_out=out_sb[:, t:t + 1])
            else:
                # loss on ACT: sum relu(xp - D)
                scr_l = work.tile([128, B], BF16, tag="scr_l")
                nc.scalar.activation(
                    scr_l, Dt, AF.Relu, bias=xp[:, t:t + 1], scale=-1.0,
                    accum_out=out_sb[:, t:t + 1])
                # count on DVE: sum (D < xp)
                scr_c = work.tile([128, B], BF16, tag="scr_c")
                nc.vector.tensor_scalar(
                    scr_c, Dt, xp[:, t:t + 1], None, op0=OP.is_lt,
                    op1=OP.add,
                    accum_out=out_sb[:, n_tiles + t:n_tiles + t + 1])

    nc.sync.dma_start(out=out_d.ap(), in_=out_sb)


def _host_prepare(labels: np.ndarray, emb: np.ndarray):
    labels = np.asarray(labels).astype(np.int64)
    emb = np.ascontiguousarray(np.asarray(emb, dtype=np.float32))
    b = labels.shape[0]
    ncls = int(labels.max()) + 1
    assert ncls <= 64

    pairs_a, pairs_p = [], []
    by_class = {}
    for i, lab in enumerate(labels.tolist()):
        by_class.setdefault(lab, []).append(i)
    for idxs in by_class.values():
        for a in idxs:
            for p in idxs:
                if a != p:
                    pairs_a.append(a)
                    pairs_p.append(p)
    np_total = len(pairs_a)
    per_core = max(1, math.ceil(np_total / NCORES))
    n_tiles = max(1, math.ceil(per_core / 128))
    npc = n_tiles * 128

    m = np.bincount(labels, minlength=ncls).astype(np.int64)
    num_valid = int((m * (m - 1) * (b - m)).sum())

    sq = (emb * emb).sum(1)
    d2 = sq[:, None] + sq[None, :] - 2.0 * (emb @ emb.T)
    neq = labels[:, None] != labels[None, :]
    assert not neq.any() or d2[neq].min() > 16.0

    embT = emb.T
    onehot = (labels[None, :] ==
              np.arange(ncls)[:, None]).astype(np.float32)
    lohx = np.zeros((96, b), np.float32)
    lohx[0:ncls, :] = onehot
    lohx[NAROW, :] = 1.0          # pairs with device ||e_a||^2 row

    in_maps = []
    for k in range(NCORES):
        a_idx = pairs_a[k * per_core:(k + 1) * per_core]
        p_idx = pairs_p[k * per_core:(k + 1) * per_core]
        nreal = len(a_idx)

        am2T = np.zeros((D, npc), np.float32)
        a_emb = np.zeros((npc, D), np.float32)
        p_emb = np.zeros((npc, D), np.float32)
        alohx = np.zeros((96, npc), np.float32)
        alohx[0:ncls, :] = BIG        # pads: BIG in every class row
        if nreal:
            ga = emb[a_idx]
            am2T[:, :nreal] = (-2.0 * ga).T
            a_emb[:nreal] = ga
            p_emb[:nreal] = emb[p_idx]
            alohx[0:ncls, :nreal] = BIG * onehot[:, a_idx]

        a_emb2 = np.ascontiguousarray(
            a_emb.reshape(n_tiles, 128, D).transpose(1, 0, 2)).reshape(128, -1)
        p_emb2 = np.ascontiguousarray(
            p_emb.reshape(n_tiles, 128, D).transpose(1, 0, 2)).reshape(128, -1)
        epad = np.zeros((128, 32), np.float32)
        epad[:, 0] = 1.0              # lhs_nn ones
        epad[:, 3] = 0.25             # lhs_na quarters
        embT_t = np.concatenate([embT, epad], axis=1)
        maskc = np.concatenate([alohx, lohx], axis=1)
        ap_c = np.concatenate([a_emb2, p_emb2], axis=1)
        in_maps.append({
            "embT": np.ascontiguousarray(embT_t).astype(BF),
            "am2T": np.ascontiguousarray(am2T).astype(BF),
            "ap_emb": np.ascontiguousarray(ap_c).astype(BF),
            "mask": np.ascontiguousarray(maskc).astype(BF),
        })
    return in_maps, n_tiles, num_valid


def kernel(labels: np.ndarray, embeddings: np.ndarray):
    global LAST_RESULT
    in_maps, n_tiles, num_valid = _host_prepare(labels, embeddings)

    if n_tiles not in _PROGRAM_CACHE:
        _PROGRAM_CACHE[n_tiles] = _build_program(n_tiles)
    nc = _PROGRAM_CACHE[n_tiles]

    res = run_bass_kernel_spmd(nc, in_maps, list(range(NCORES)), trace=TRACE)
    LAST_RESULT = res

    outs = np.stack([np.asarray(r["out"], np.float64) for r in res.results])
    nact = min(2, n_tiles)
    # loss: tiles 0..1 accumulate -sum relu (DVE), 2.. +sum relu (ACT);
    # counts: tiles 0..1 sign-coded (2c-512), the rest direct
    s_sum = (-outs[:, :, 0:nact].sum()) + outs[:, :, nact:n_tiles].sum()
    csign = outs[:, :, n_tiles:n_tiles + nact]
    c_sum = ((csign + 512.0) / 2.0).sum() + \
        outs[:, :, n_tiles + nact:2 * n_tiles].sum()
    loss = np.float32(s_sum / (c_sum + 1e-16))
    frac = np.float32(c_sum / (num_valid + 1e-16))
    return (np.asarray(loss, np.float32), np.asarray(frac, np.float32))
